# revision 69
# baseline (speedup 1.0000x reference)
"""Trainium2 Bass kernel for nn_EncoderDecoderTransformer (sparse kNN encoder attention).

Sharding: data-parallel over batch (4 batches x 2 cores) with each pair of cores
splitting the sequence dimension (512 tokens each). Per attention, the pair
exchanges the LN output h (0.5 MB bf16) via AllGather over replica groups
[[0,1],[2,3],[4,5],[6,7]] -- fired right after the LN, overlapped with the Q
projection -- and each core then computes K/V for the full 1024-token sequence
locally (K tiles 1-3 and V tiles 2-7 are deferred thunks woven into the
attention pipeline, one per step). The encoder output is exchanged once and
every decoder layer projects its own cross-attention K/V from it.

Layouts (per core):
  - Activations feature-major: x^T stored as one [128, 4, 512] f32 tile;
    residual adds also emit the bf16 copy + squares + sum-matmuls per chunk so
    the next LN's stats overlap the producer's tail.
  - LN row-pair staging (rstd/cro, 1/den) lives at partitions 0 and 32 of
    33-row tiles (engine partition bases must be 32-aligned); selector/ones
    matmuls broadcast them across partitions in bf16.
  - Q^T/K^T feature-major (head h lives in rows [64*(h%2):...] of ptile h//2).
  - V token-major (128 tokens, 8 heads, 65) with a constant-1 column per head so
    the AV matmul also produces the softmax denominator in psum row 64.
  - Scores computed transposed: S^T = K^T.T @ Q^T (keys on partitions), the two
    heads of a pair sharing one [128, 1024] PSUM tile -> one exp slice on the
    Scalar engine and one mask multiply on Vector (causal masks are read twice
    through a stride-0 AP). The attention is a single software-pipelined stream
    over (head-pair, key-tile): AV lags scores by 2 steps and each pair's
    normalization is emitted after the next pair's first scores so it never
    blocks the PE queue.
  - The activation-table view is trimmed so ln/exp/square/identity all resolve
    to the natural_log_exp_and_others set (only gelu switches tables).
  - kNN mask: s'_qk = 2 x_q.x_k - |x_k|^2 orders like -distance. The 17th
    largest per row (self is always rank 1) is the inclusion threshold; computed
    with the DVE max8/match_replace top-k primitives (IEEE-exact fp32).
"""

import os
import numpy as np
import ml_dtypes

BF16 = ml_dtypes.bfloat16
F8 = ml_dtypes.float8_e4m3

D, F, H, NE, ND, KNN = 512, 2048, 8, 4, 4, 16
B, LE, LD = 4, 1024, 1024
DH = D // H
NCORE = 8
P = 128
TOWN = 512          # tokens owned per core
NDT = D // P        # 4 feature tiles
NKT = LE // P       # 8 key tiles
NEG = -1e30
EPS = 1e-5
PAIRS = [[0, 1], [2, 3], [4, 5], [6, 7]]

_CACHE = {}


def build(n_enc=NE, n_dec=ND):
    from contextlib import ExitStack

    import concourse.bacc as bacc
    import concourse.tile as tile
    import concourse.mybir as mybir

    f32 = mybir.dt.float32
    bf16 = mybir.dt.bfloat16
    AF = mybir.ActivationFunctionType
    OP = mybir.AluOpType

    nc = bacc.Bacc("TRN2", target_bir_lowering=False, debug=False, num_devices=NCORE)

    # Steer the activation-table pass: by default exp binds to the
    # `exp_and_others` set and ln to `natural_log`, so every layer norm
    # (ln then exp) pays two serialized ~1.3us ACT_TABLE_LOADs. Removing
    # exp/ln from those sets in the bass-side view leaves
    # `natural_log_exp_and_others` as the only set providing either, so
    # ln, exp (incl. softmax), square and identity all share one resident
    # table and only gelu forces a switch. Indices are unchanged, so the
    # emitted act_func_set_id still refers to the real act_info.json sets.
    import concourse.hw_specs as hw_specs

    tabs = hw_specs.get_activation_tables(nc.m.arch)
    AFT = mybir.ActivationFunctionType
    if AFT.Exp in tabs.get("exp_and_others", set()):
        tabs["exp_and_others"].discard(AFT.Exp)
        tabs["natural_log"].discard(AFT.Ln)

    # ---- I/O ----
    def din(name, shape, dt=f32):
        return nc.dram_tensor(name, shape, dt, kind="ExternalInput")

    x0T = din("x0T", [NDT, P, TOWN])
    y0T = din("y0T", [NDT, P, TOWN])
    xq2_d = din("xq2", [TOWN, 3])       # 2*xyz for own tokens
    xq2row_d = din("xq2row", [3, TOWN])  # same, transposed
    xkn_d = din("xkn", [LE, 4])          # [xyz, |xyz|^2] all tokens
    xrow_d = din("xrow", [4, LE])        # same, transposed
    bosrow = din("bosrow", [1, TOWN])
    causal_in = din("causal", [NKT, P, TOWN], bf16)

    ew_qkv = din("ew_qkv", [NE, D, 3 * D], bf16)
    ew_out = din("ew_out", [NE, D, D], bf16)
    ew_f1 = din("ew_f1", [NE, D, F], bf16)
    ew_f2 = din("ew_f2", [NE, F, D], bf16)
    eb_qkv = din("eb_qkv", [NE, 3 * D, 1])
    eb_out = din("eb_out", [NE, D, 1])
    eb_f1 = din("eb_f1", [NE, F, 1])
    eb_f2 = din("eb_f2", [NE, D, 1])

    dw_saqkv = din("dw_saqkv", [ND, D, 3 * D], bf16)
    db_saqkv = din("db_saqkv", [ND, 3 * D, 1])
    dw_saout = din("dw_saout", [ND, D, D], bf16)
    db_saout = din("db_saout", [ND, D, 1])
    dw_caqkv = din("dw_caqkv", [ND, D, 3 * D], bf16)
    db_caqkv = din("db_caqkv", [ND, 3 * D, 1])
    dw_caout = din("dw_caout", [ND, D, D], bf16)
    db_caout = din("db_caout", [ND, D, 1])
    dw_f1 = din("dw_f1", [ND, D, F], bf16)
    db_f1 = din("db_f1", [ND, F, 1])
    dw_f2 = din("dw_f2", [ND, F, D], bf16)
    db_f2 = din("db_f2", [ND, D, 1])
    eb_qkv_bf = din("eb_qkv_bf", [NE, 3 * D, 1], bf16)
    db_saqkv_bf = din("db_saqkv_bf", [ND, 3 * D, 1], bf16)
    db_caqkv_bf = din("db_caqkv_bf", [ND, 3 * D, 1], bf16)

    enc_part = nc.dram_tensor("enc_part", [NDT, P, TOWN], f32, kind="ExternalOutput")
    dec_part = nc.dram_tensor("dec_part", [NDT, P, TOWN], f32, kind="ExternalOutput")
    n_dbg = int(os.environ.get("KQ_DEBUG", "0"))
    dbg_t = None
    if n_dbg:
        dbg_t = nc.dram_tensor("dbg", [n_dbg, P, NDT * TOWN], f32, kind="ExternalOutput")

    with tile.TileContext(nc) as tc, ExitStack() as ctx:
        ep = ctx.enter_context

        pc = ep(tc.tile_pool(name="pc", bufs=1))
        p_allow = ep(tc.tile_pool(name="p_allow", bufs=8))
        p_causal = ep(tc.tile_pool(name="p_causal", bufs=8))
        ps_s = ep(tc.tile_pool(name="ps_s", bufs=2, space="PSUM"))
        ps_o = ep(tc.tile_pool(name="ps_o", bufs=2, space="PSUM"))
        ps_mm = ep(tc.tile_pool(name="ps_mm", bufs=2, space="PSUM"))
        p_dram = ep(tc.tile_pool(name="p_dram", bufs=2, space="DRAM"))

        # ---- constants ----
        ones_col_bf = pc.tile([P, 1], bf16)
        nc.vector.memset(ones_col_bf, 1.0)
        ones_row = pc.tile([1, P], f32)
        nc.vector.memset(ones_row, 1.0)
        ones_row512_bf = pc.tile([1, TOWN], bf16)
        nc.vector.memset(ones_row512_bf, 1.0)
        eps_sb = pc.tile([1, 1], f32)
        nc.vector.memset(eps_sb, EPS)
# row pairs live at partitions 0 and 32 (engine partition offsets must be
        # 32-aligned); the zeroed selector rows in between nullify garbage rows
        SR = 33
        sel2 = pc.tile([SR, P], bf16)    # denominator broadcast selector
        nc.vector.memset(sel2, 0.0)
        nc.vector.memset(sel2[0:1, 0:64], 1.0)
        nc.vector.memset(sel2[32:33, 64:128], 1.0)
        ones33 = pc.tile([SR, P], bf16)  # all-ones rows at any 32-aligned base
        nc.vector.memset(ones33, 1.0)
        # persistent zeroed row-pair staging tiles (rows 1..31 stay zero so
        # the selector matmuls never see NaN garbage)
        rec2f_c = pc.tile([SR, TOWN], f32)
        nc.vector.memset(rec2f_c, 1.0)
        den2_c = pc.tile([SR, TOWN], f32)
        nc.vector.memset(den2_c, 1.0)
        rc2_c = pc.tile([SR, TOWN], bf16)
        nc.vector.memset(rc2_c, 0.0)

        bos_sb = pc.tile([1, TOWN], f32)
        nc.sync.dma_start(out=bos_sb, in_=bosrow[:, :])

        causal_sb = []
        for kt in range(NKT):
            t = p_causal.tile([P, TOWN], bf16, tag="causal")
            nc.sync.dma_start(out=t, in_=causal_in[kt])
            causal_sb.append(t)

        def build_mask():
            # s'_qk = 2 x_q . x_k - |x_k|^2 computed with IEEE-exact fp32 DVE ops
            # (the PE fp32 matmul is not exact fp32 and flips kNN boundary choices).
            # Both layouts use the same per-element op chain => bit-identical values.
            import concourse.bass as cbass

            def bcast_rows(dram_row_ap, pool, n_free, tag):
                # (n_free,) DRAM row -> (P, n_free) SBUF tile, replicated across partitions
                t = pool.tile([P, n_free], f32, tag=tag)
                src_ap = cbass.AP(
                    tensor=dram_row_ap.tensor, offset=dram_row_ap.offset,
                    ap=[[0, P]] + list(dram_row_ap.ap),
                )
                nc.sync.dma_start(out=t, in_=src_ap)
                return t

            allow_sb = []
            with tc.tile_pool(name="p_mask", bufs=2) as p_mask, \
                 tc.tile_pool(name="p_mbc", bufs=1) as p_mbc, \
                 tc.tile_pool(name="p_m8", bufs=8) as p_m8:
                bcx = []
                for c in range(4):
                    t = bcast_rows(xrow_d[c], p_mbc, LE, tag=f"bcx{c}")
                    bcx.append(t)
                tcol_dram = p_dram.tile([4, P, 1], f32, tag="tcol")
                for qt in range(4):
                    xqc = p_m8.tile([P, 3], f32, tag="xqc")
                    nc.sync.dma_start(out=xqc, in_=xq2_d[qt * P:(qt + 1) * P, :])
                    s0 = p_mask.tile([P, LE], f32, tag="s")
                    nc.vector.tensor_scalar(s0, bcx[0], xqc[:, 0:1], None, op0=OP.mult)
                    s1 = p_mask.tile([P, LE], f32, tag="s")
                    nc.vector.scalar_tensor_tensor(s1, bcx[1], xqc[:, 1:2], s0, OP.mult, OP.add)
                    s2 = p_mask.tile([P, LE], f32, tag="s")
                    nc.vector.scalar_tensor_tensor(s2, bcx[2], xqc[:, 2:3], s1, OP.mult, OP.add)
                    s3 = p_mask.tile([P, LE], f32, tag="s")
                    nc.vector.tensor_tensor(s3, s2, bcx[3], OP.subtract)
                    m8 = p_m8.tile([P, 8], f32, tag="m8")
                    nc.vector.max(m8, s3)
                    s4 = p_mask.tile([P, LE], f32, tag="s")
                    nc.vector.match_replace(s4, m8, s3, NEG)
                    m8b = p_m8.tile([P, 8], f32, tag="m8")
                    nc.vector.max(m8b, s4)
                    s5 = p_mask.tile([P, LE], f32, tag="s")
                    nc.vector.match_replace(s5, m8b, s4, NEG)
                    m8c = p_m8.tile([P, 8], f32, tag="m8")
                    nc.vector.max(m8c, s5)
                    # rank-17 value (16 NN + self) is the inclusion threshold
                    nc.sync.dma_start(out=tcol_dram[qt], in_=m8c[:, 0:1])
                t_row = pc.tile([1, TOWN], f32)
                nc.sync.dma_start(
                    out=t_row, in_=tcol_dram.rearrange("a p one -> one (a p)")
                )
                t2 = pc.tile([1, TOWN], f32)
                nc.vector.tensor_tensor(t2, t_row, bos_sb, OP.min)
                t2_dram = p_dram.tile([1, TOWN], f32, tag="t2d")
                nc.sync.dma_start(out=t2_dram, in_=t2)
                t_bc = bcast_rows(t2_dram[0], pc, TOWN, tag="t_bc")
                bq = []
                for c in range(3):
                    t = bcast_rows(xq2row_d[c], p_mbc, TOWN, tag=f"bq{c}")
                    bq.append(t)
                for kt in range(NKT):
                    xkc = p_m8.tile([P, 4], f32, tag="xkc")
                    nc.sync.dma_start(out=xkc, in_=xkn_d[kt * P:(kt + 1) * P, :])
                    u0 = p_mask.tile([P, TOWN], f32, tag="st")
                    nc.vector.tensor_scalar(u0, bq[0], xkc[:, 0:1], None, op0=OP.mult)
                    u1 = p_mask.tile([P, TOWN], f32, tag="st")
                    nc.vector.scalar_tensor_tensor(u1, bq[1], xkc[:, 1:2], u0, OP.mult, OP.add)
                    u2 = p_mask.tile([P, TOWN], f32, tag="st")
                    nc.vector.scalar_tensor_tensor(u2, bq[2], xkc[:, 2:3], u1, OP.mult, OP.add)
                    u3 = p_mask.tile([P, TOWN], f32, tag="st")
                    nc.vector.tensor_scalar(u3, u2, xkc[:, 3:4], None, op0=OP.subtract)
                    # duplicated [q | q] layout so a head-pair shares one multiply
                    al = p_allow.tile([P, 2 * TOWN], bf16, tag="allow")
                    nc.vector.tensor_tensor(al[:, 0:TOWN], u3, t_bc, OP.is_ge)
                    nc.vector.tensor_copy(al[:, TOWN:2 * TOWN], al[:, 0:TOWN])
                    allow_sb.append(al)
                nc.vector.memset(allow_sb[0][0:1, :], 1.0)  # BOS key allowed for all q
            return allow_sb

        allow_sb = build_mask()

        dbg_i = [0]
        dbg_stage = None
        if dbg_t is not None:
            dbg_stage = pc.tile([P, TOWN], f32)
            nc.vector.memset(dbg_stage, 0.0)

        def dbg(ap, np_, nf):
            # dump a [np_, nf] view (any dtype) as f32 for offline comparison
            if dbg_t is None or dbg_i[0] >= n_dbg:
                return
            for c in range((nf + TOWN - 1) // TOWN):
                w = min(TOWN, nf - c * TOWN)
                nc.vector.tensor_copy(dbg_stage[:np_, :w], ap[:, c * TOWN:c * TOWN + w])
                nc.sync.dma_start(
                    out=dbg_t[dbg_i[0]][:np_, c * TOWN:c * TOWN + w],
                    in_=dbg_stage[:np_, :w],
                )
            dbg_i[0] += 1

        # ================= helpers =================
        def load_bias(bias_ap, n, tag):
            # all n per-partition bias columns of a projection in one DMA
            t = p_bias.tile([P, n], f32, tag=tag)
            nc.sync.dma_start(
                out=t, in_=bias_ap.rearrange("(m p) one -> p (m one)", p=P)
            )
            return t

        def load_w(pool, dram_ap, kchunks, cols, tag, dt=bf16):
            t = pool.tile([P, kchunks, cols], bf16 if dt is None else dt, tag=tag)
            nc.sync.dma_start(
                out=t, in_=dram_ap.rearrange("(kc p) m -> p kc m", p=P)
            )
            return t

        def new_stats():
            # bf16 copy + squares of the residual stream, filled per chunk by
            # the producer (residual adds) so LN stats overlap the tail
            return {
                "xb4": p_lnsq.tile([P, NDT, TOWN], bf16, tag="lnxb", name="xb4"),
                "sq4": p_lnsq.tile([P, NDT, TOWN], bf16, tag="lnsq", name="sq4"),
            }

        def emit_stats_chunk(st, m, x_chunk):
            nc.vector.tensor_copy(st["xb4"][:, m, :], x_chunk)
            nc.scalar.activation(st["sq4"][:, m, :], x_chunk, AF.Square)

        def layer_norm(x4, out_dt, out_pool, out_tag, stats=None):
            if stats is None:
                stats = new_stats()
                for dt in range(NDT):
                    emit_stats_chunk(stats, dt, x4[:, dt, :])
            sq4, xb4 = stats["sq4"], stats["xb4"]
            ps_mean = ps_mm.tile([1, TOWN], f32, tag="mm")
            for dt in range(NDT):
                nc.tensor.matmul(ps_mean, ones_col_bf, xb4[:, dt, :], start=dt == 0, stop=dt == 3)
            ps_sq = ps_mm.tile([1, TOWN], f32, tag="mm")
            for dt in range(NDT):
                nc.tensor.matmul(ps_sq, ones_col_bf, sq4[:, dt, :], start=dt == 0, stop=dt == 3)
            mu = p_small.tile([1, TOWN], f32, tag="sm")
            nc.vector.tensor_single_scalar(mu, ps_mean, 1.0 / D, OP.mult)
            musq = p_small.tile([1, TOWN], f32, tag="sm")
            nc.vector.tensor_tensor(musq, mu, mu, OP.mult)
            var = p_small.tile([1, TOWN], f32, tag="sm")
            nc.vector.scalar_tensor_tensor(var, ps_sq, 1.0 / D, musq, OP.mult, OP.subtract)
            lnv = p_small.tile([1, TOWN], f32, tag="sm")
            nc.scalar.activation(lnv, var, AF.Ln, bias=eps_sb)
            nc.scalar.activation(rc2_c[0:1, :], lnv, AF.Exp, scale=-0.5)
            nc.vector.scalar_tensor_tensor(rc2_c[32:33, :], mu, -1.0, rc2_c[0:1, :], OP.mult, OP.mult)
            ps_a = ps_mm.tile([P, TOWN], f32, tag="mm")
            nc.tensor.matmul(ps_a, ones33[0:1, :], rc2_c[0:1, :], start=True, stop=True)
            ps_c = ps_mm.tile([P, TOWN], f32, tag="mm")
            nc.tensor.matmul(ps_c, ones33[32:33, :], rc2_c[32:33, :], start=True, stop=True)
            a_sb = p_lnac.tile([P, TOWN], bf16, tag="lna")
            nc.vector.tensor_copy(a_sb, ps_a)
            c_sb = p_lnac.tile([P, TOWN], bf16, tag="lnc")
            nc.vector.tensor_copy(c_sb, ps_c)
            h4 = out_pool.tile([P, NDT, TOWN], out_dt, tag=out_tag)
            hx = xb4 if out_dt == bf16 else x4  # final f32 LNs keep full precision
            for dt in range(NDT):
                nc.vector.tensor_tensor(h4[:, dt, :], hx[:, dt, :], a_sb, OP.mult)
                nc.vector.tensor_tensor(h4[:, dt, :], h4[:, dt, :], c_sb, OP.add)
            return h4

        def proj_fm(w_sb, col_off, n_m, h4, bt, bt_off, out_pool, out_tag, out_dt=bf16):
            """Feature-major projection; per-partition bias applied on eviction."""
            outs = []
            for m in range(n_m):
                ps = ps_mm.tile([P, TOWN], f32, tag="mm")
                for kc in range(NDT):
                    nc.tensor.matmul(
                        ps, w_sb[:, kc, col_off + m * P:col_off + (m + 1) * P],
                        h4[:, kc, :], start=kc == 0, stop=kc == NDT - 1,
                    )
                o = out_pool.tile([P, TOWN], out_dt, tag=out_tag)
                nc.scalar.activation(o, ps, AF.Identity, bias=bt[:, bt_off + m:bt_off + m + 1])
                outs.append(o)
            return outs

        def h_allgather(h4, uniq):
            """Exchange the LN output within the pair (half the bytes of a
            K+V exchange, and it fires right after the LN, before any
            projection). Each core then computes K/V for the full sequence."""
            bin_h = p_dram.tile([P, NDT * TOWN], bf16, tag=f"hin{uniq}")
            for dt in range(NDT):
                # per-chunk staging DMAs overlap the LN eviction tail
                nc.sync.dma_start(
                    out=bin_h[:, dt * TOWN:(dt + 1) * TOWN], in_=h4[:, dt, :])
            bout = p_dram.tile([2, P, NDT * TOWN], bf16, tag=f"hout{uniq}")
            nc.gpsimd.collective_compute(
                "AllGather", OP.bypass, replica_groups=PAIRS,
                ins=[bin_h[:].opt()], outs=[bout[:].opt()],
            )
            return bout

        def load_hall(bout):
            # [feat-chunk part, kc, rank, tok]; rank slot = global seq half
            h_all = p_hall.tile([P, NDT, 2, TOWN], bf16, tag="hall")
            for r in range(2):
                nc.sync.dma_start(
                    out=h_all[:, :, r, :],
                    in_=bout[r].rearrange("p (kc t) -> p kc t", kc=NDT),
                )
            return h_all

        def proj_k_tile(w_sb, col_off, h_all, bias_bt, bias_off, m, on_vector=False):
            """K^T feature tile m (both seq halves) -> [P, 2, TOWN] bf16.
            on_vector: evict on DVE (used for weave thunks so the Scalar
            engine's softmax exp stream is not interrupted)."""
            kt_ = p_kv.tile([P, 2, TOWN], bf16, tag="ksb", name=f"k{m}")
            bcol = bias_bt[:, bias_off + m:bias_off + m + 1]
            for r in range(2):
                ps = ps_mm.tile([P, TOWN], f32, tag="mm")
                for kc in range(NDT):
                    nc.tensor.matmul(
                        ps, w_sb[:, kc, col_off + m * P:col_off + (m + 1) * P],
                        h_all[:, kc, r, :], start=kc == 0, stop=kc == NDT - 1,
                    )
                if on_vector:
                    nc.vector.tensor_scalar(kt_[:, r, :], ps, bcol, None, op0=OP.add)
                else:
                    nc.scalar.activation(kt_[:, r, :], ps, AF.Identity, bias=bcol)
            return kt_

        def proj_v_tile(w_sb, col_off, h_all, brow512, kt):
            """V token-major key-tile kt -> [P(tokens), H, 65] bf16 with the
            ones column; bias via rank-1 bf16 matmul."""
            r, tc_ = kt // 4, kt % 4
            ps = ps_mm.tile([P, TOWN], f32, tag="mm")
            for kc in range(4):
                nc.tensor.matmul(
                    ps, h_all[:, kc, r, tc_ * P:(tc_ + 1) * P],
                    w_sb[:, kc, col_off:col_off + D],
                    start=kc == 0, stop=False,
                )
            nc.tensor.matmul(ps, ones_row512_bf[:, 0:P], brow512, start=False, stop=True)
            vt = p_v.tile([P, H, 65], bf16, tag="vsb", name=f"v{kt}")
            nc.vector.tensor_copy(vt[:, :, 0:64], ps.rearrange("p (h d) -> p h d", h=H))
            nc.vector.memset(vt[:, :, 64:65], 1.0)
            return vt

        def load_brow(bias_ap, bias_off):
            brow512 = p_bias.tile([1, TOWN], bf16, tag="brow512")
            nc.sync.dma_start(
                out=brow512,
                in_=bias_ap[bias_off:bias_off + D, :].rearrange("a b -> b a"),
            )
            return brow512

        def make_kv(w_sb, k_off, v_off, h_all, bias_bt, bias_bf_ap, k_boff, v_boff):
            """Emit K tile 0 and V tiles 0-1 up front; defer the rest as weave
            thunks consumed one per attention pipeline step (deadlines with
            LAG=2: V[kt] by step kt+2, K[m] by step 8m)."""
            brow512 = load_brow(bias_bf_ap, v_boff)
            Ks = [None] * 4
            Vs = [None] * NKT
            Ks[0] = proj_k_tile(w_sb, k_off, h_all, bias_bt, k_boff, 0)
            Vs[0] = proj_v_tile(w_sb, v_off, h_all, brow512, 0)
            Vs[1] = proj_v_tile(w_sb, v_off, h_all, brow512, 1)
            weave = [
                ("v", kt, lambda kt=kt: proj_v_tile(w_sb, v_off, h_all, brow512, kt))
                for kt in range(2, NKT)
            ] + [
                ("k", m, lambda m=m: proj_k_tile(w_sb, k_off, h_all, bias_bt, k_boff, m, on_vector=True))
                for m in range(1, 4)
            ]
            return Ks, Vs, weave

        def attn_norm(psO, dbg_attn=False):
            # rec = 1/den (bf16), broadcast rows via one K=33 matmul,
            # multiply out of PSUM
            if dbg_attn:
                dbg(psO[0], 65, TOWN)
            for j in range(2):
                nc.vector.tensor_copy(den2_c[32 * j:32 * j + 1, :], psO[j][64:65, :])
            nc.vector.reciprocal_approx_fast(rec2f_c, den2_c)
            rec2 = p_rec.tile([SR, TOWN], bf16, tag="rec2")
            nc.vector.tensor_copy(rec2, rec2f_c)
            psB = ps_mm.tile([P, TOWN], f32, tag="mm")
            nc.tensor.matmul(psB, sel2, rec2, start=True, stop=True)
            bc = p_bc.tile([P, TOWN], bf16, tag="bc")
            nc.vector.tensor_copy(bc, psB)
            if dbg_attn:
                dbg(rec2, SR, TOWN)
                dbg(bc, P, TOWN)
            ot = p_ot.tile([P, TOWN], bf16, tag="ot")
            for j in range(2):
                rows = slice(j * 64, (j + 1) * 64)
                nc.vector.tensor_tensor(ot[rows, :], psO[j][0:64, :], bc[rows, :], OP.mult)
            return ot

        LAG = 3

        def rep2(ap):
            # view a [P, TOWN] tile as [P, 2, TOWN] via a stride-0 middle dim
            import concourse.bass as cbass
            a = ap.opt() if hasattr(ap, "opt") else ap
            return cbass.AP(
                tensor=a.tensor, offset=a.offset,
                ap=[list(a.ap[0]), [0, 2]] + [list(x) for x in a.ap[1:]],
            )

        def attention(Qs, Ks, Vs, mask_tiles, mask_dup, dbg_attn=False, weave=()):
            """Single software-pipelined stream over (head-pair, key-tile):
            AV lags scores by LAG steps and each head-pair's normalization is
            emitted after the next pair's first scores (no PE-queue blocking).
            Deferred K/V projection thunks (weave) are consumed one per step.
            mask_dup=True: mask tiles are [P, 2*TOWN] (head-pair duplicated);
            mask_dup=False: [P, TOWN], applied per head half."""
            OTs = [None] * 4
            psOs = [None] * 4
            es = {}
            weave = list(weave)
            TOT = 4 * NKT
            for gs in range(TOT + LAG):
                if weave:
                    kind, idx, fn = weave.pop(0)
                    (Ks if kind == "k" else Vs)[idx] = fn()
                if gs < TOT:
                    hp, kt = divmod(gs, NKT)
                    r, c = kt // 4, kt % 4
                    psS = ps_s.tile([P, 2 * TOWN], f32, tag="pss")
                    for j in range(2):
                        rows = slice(j * 64, (j + 1) * 64)
                        nc.tensor.matmul(
                            psS[:, j * TOWN:(j + 1) * TOWN],
                            Ks[hp][rows, r, c * P:(c + 1) * P], Qs[hp][rows, :],
                            start=True, stop=True,
                        )
                    e = p_e.tile([P, 2 * TOWN], bf16, tag="e")
                    nc.scalar.activation(e, psS, AF.Exp, scale=0.125)
                    if mask_tiles is None:
                        es[gs] = e
                    else:
                        em = p_e.tile([P, 2 * TOWN], bf16, tag="em")
                        if mask_dup:
                            nc.vector.tensor_tensor(em, e, mask_tiles[kt], OP.mult)
                        else:
                            nc.vector.tensor_tensor(em, e, rep2(mask_tiles[kt]), OP.mult)
                        es[gs] = em
                    if dbg_attn and gs == 0:
                        dbg(es[0], P, 2 * TOWN)
                gsA = gs - LAG
                if gsA >= 0:
                    hpA, ktA = divmod(gsA, NKT)
                    if ktA == 0:
                        psOs[hpA] = [
                            ps_o.tile([65, TOWN], f32, tag="pso", name=f"psO{hpA}_{j}")
                            for j in range(2)
                        ]
                    for j in range(2):
                        head = 2 * hpA + j
                        nc.tensor.matmul(
                            psOs[hpA][j], Vs[ktA][:, head, :],
                            es[gsA][:, j * TOWN:(j + 1) * TOWN],
                            start=ktA == 0, stop=ktA == NKT - 1,
                        )
                    del es[gsA]
                    if ktA == NKT - 1:
                        OTs[hpA] = attn_norm(psOs[hpA], dbg_attn and hpA == 0)
                        psOs[hpA] = None
            return OTs

        def proj_residual(w_sb, col_off, n_k, rhs, bias_ap, x4):
            nx4 = p_x.tile([P, NDT, TOWN], f32, tag="x")
            st = new_stats()
            bt = load_bias(bias_ap, NDT, "bias4")
            for m in range(NDT):
                ps = ps_mm.tile([P, TOWN], f32, tag="mm")
                for kc in range(n_k):
                    nc.tensor.matmul(
                        ps, w_sb[:, kc, col_off + m * P:col_off + (m + 1) * P],
                        rhs[kc], start=kc == 0, stop=kc == n_k - 1,
                    )
                nc.vector.scalar_tensor_tensor(nx4[:, m, :], ps, bt[:, m:m + 1], x4[:, m, :], OP.add, OP.add)
                emit_stats_chunk(st, m, nx4[:, m, :])
            return nx4, st

        def ffn(w1_ap, w2_ap, b1_ap, b2_ap, h4, x4):
            gs = []
            bt1 = load_bias(b1_ap, F // P, "bias16")
            bt2 = load_bias(b2_ap, NDT, "bias4")
            for m in range(F // P):
                w1m = p_w1.tile([P, NDT, P], bf16, tag="wf1")
                nc.sync.dma_start(
                    out=w1m,
                    in_=w1_ap[:, m * P:(m + 1) * P].rearrange("(kc p) m -> p kc m", p=P),
                )
                ps = ps_mm.tile([P, TOWN], f32, tag="mm")
                for kc in range(NDT):
                    nc.tensor.matmul(
                        ps, w1m[:, kc, :], h4[:, kc, :],
                        start=kc == 0, stop=kc == NDT - 1,
                    )
                g = p_g.tile([P, TOWN], bf16, tag="g")
                nc.scalar.activation(g, ps, AF.Gelu, bias=bt1[:, m:m + 1])
                gs.append(g)
            nx4 = p_x.tile([P, NDT, TOWN], f32, tag="x")
            st = new_stats()
            for m in range(NDT):
                w2m = p_w2.tile([P, F // P, P], bf16, tag="wf2")
                nc.sync.dma_start(
                    out=w2m,
                    in_=w2_ap[:, m * P:(m + 1) * P].rearrange("(kc p) c -> p kc c", p=P),
                )
                ps2 = ps_mm.tile([P, TOWN], f32, tag="mm")
                for kc in range(F // P):
                    nc.tensor.matmul(
                        ps2, w2m[:, kc, :], gs[kc],
                        start=kc == 0, stop=kc == F // P - 1,
                    )
                nc.vector.scalar_tensor_tensor(nx4[:, m, :], ps2, bt2[:, m:m + 1], x4[:, m, :], OP.add, OP.add)
                emit_stats_chunk(st, m, nx4[:, m, :])
            return nx4, st

        p_x = ep(tc.tile_pool(name="p_x", bufs=2))
        p_h = ep(tc.tile_pool(name="p_h", bufs=2))
        p_q = ep(tc.tile_pool(name="p_q", bufs=4 if dbg_t is not None else 5))
        p_kv = ep(tc.tile_pool(name="p_kv", bufs=4))
        p_v = ep(tc.tile_pool(name="p_v", bufs=8))
        p_hall = ep(tc.tile_pool(name="p_hall", bufs=1))
        p_eoball = ep(tc.tile_pool(name="p_eoball", bufs=1))
        p_ot = ep(tc.tile_pool(name="p_ot", bufs=4))
        p_e = ep(tc.tile_pool(name="p_e", bufs=4))
        p_g = ep(tc.tile_pool(name="p_g", bufs=16))
        p_lnsq = ep(tc.tile_pool(name="p_lnsq", bufs=1))
        p_lnac = ep(tc.tile_pool(name="p_lnac", bufs=1))
        p_bc = ep(tc.tile_pool(name="p_bc", bufs=2))
        p_small = ep(tc.tile_pool(name="p_small", bufs=5))
        p_rec = ep(tc.tile_pool(name="p_rec", bufs=2))
        p_bias = ep(tc.tile_pool(name="p_bias", bufs=2))
        p_eo = ep(tc.tile_pool(name="p_eo", bufs=1))
        p_eob = ep(tc.tile_pool(name="p_eob", bufs=1))
        p_w1 = ep(tc.tile_pool(name="p_w1", bufs=2 if dbg_t is not None else 3))
        p_w2 = ep(tc.tile_pool(name="p_w2", bufs=2))
        p_wqkv = ep(tc.tile_pool(name="p_wqkv", bufs=1))
        p_wout = ep(tc.tile_pool(name="p_wout", bufs=2))

        # ================= encoder =================
        x4 = p_x.tile([P, NDT, TOWN], f32, tag="x")
        nc.sync.dma_start(out=x4, in_=x0T[:].rearrange("a p t -> p a t"))

        allow_sb = None
        for l in range(n_enc):
            wqkv = load_w(p_wqkv, ew_qkv[l], NDT, 3 * D, "wqkv")
            wout = load_w(p_wout, ew_out[l], NDT, D, "wout")

            h4 = layer_norm(x4, bf16, p_h, "h")
            bout_h = h_allgather(h4, f"e{l}")
            bqkv = load_bias(eb_qkv[l], 12, "bias12")
            Qs = proj_fm(wqkv, 0, 4, h4, bqkv, 0, p_q, "q")
            h_all = load_hall(bout_h)
            Ks, Vs, weave = make_kv(wqkv, D, 2 * D, h_all, bqkv, eb_qkv_bf[l], 4, 2 * D)
            if l == 0:
                dbg(h4.rearrange("p a t -> p (a t)"), P, NDT * TOWN)
                dbg(allow_sb[0], P, 2 * TOWN)
                dbg(Ks[0][:, 0, :], P, TOWN)
                dbg(Qs[0], P, TOWN)
                dbg(Ks[0].rearrange("p a t -> p (a t)"), P, 2 * TOWN)
                dbg(Vs[0].rearrange("p h d -> p (h d)"), P, H * 65)
            OTs = attention(Qs, Ks, Vs, allow_sb, True, dbg_attn=(l == 0), weave=weave)
            if l == 0:
                dbg(OTs[0], P, TOWN)
            x4, xst = proj_residual(wout, 0, NDT, OTs, eb_out[l], x4)
            if l == 0:
                dbg(x4.rearrange("p a t -> p (a t)"), P, NDT * TOWN)
            h4 = layer_norm(x4, bf16, p_h, "h", stats=xst)
            x4, xst = ffn(ew_f1[l], ew_f2[l], eb_f1[l], eb_f2[l], h4, x4)
            if l == 0:
                dbg(x4.rearrange("p a t -> p (a t)"), P, NDT * TOWN)

        eof = layer_norm(x4, f32, p_eo, "eof", stats=xst)
        eob = p_eob.tile([P, NDT, TOWN], bf16, tag="eob")
        nc.vector.tensor_copy(eob, eof)
        for dt in range(NDT):
            nc.sync.dma_start(out=enc_part[dt], in_=eof[:, dt, :])

        # exchange the (normed, bf16) encoder output once; every decoder
        # layer projects its own cross-attention K/V for the full sequence
        bout_e = h_allgather(eob, "eob")
        eob_all = p_eoball.tile([P, NDT, 2, TOWN], bf16, tag="eoball")
        for r in range(2):
            nc.sync.dma_start(
                out=eob_all[:, :, r, :],
                in_=bout_e[r].rearrange("p (kc t) -> p kc t", kc=NDT),
            )

        # ================= decoder =================
        y4 = p_x.tile([P, NDT, TOWN], f32, tag="x")
        nc.sync.dma_start(out=y4, in_=y0T[:].rearrange("a p t -> p a t"))

        for l in range(n_dec):
            wqkv = load_w(p_wqkv, dw_saqkv[l], NDT, 3 * D, "wqkv")
            wout = load_w(p_wout, dw_saout[l], NDT, D, "wout")

            # self-attention (causal)
            h4 = layer_norm(y4, bf16, p_h, "h")
            bout_h = h_allgather(h4, f"d{l}")
            bqkv = load_bias(db_saqkv[l], 12, "bias12")
            Qs = proj_fm(wqkv, 0, 4, h4, bqkv, 0, p_q, "q")
            h_all = load_hall(bout_h)
            Ks, Vs, weave = make_kv(wqkv, D, 2 * D, h_all, bqkv, db_saqkv_bf[l], 4, 2 * D)
            OTs = attention(Qs, Ks, Vs, causal_sb, False, weave=weave)
            y4, yst = proj_residual(wout, 0, NDT, OTs, db_saout[l], y4)

            # cross-attention (no mask)
            wkv = p_wqkv.tile([P, NDT, 2 * D], bf16, tag="wqkv")
            nc.sync.dma_start(
                out=wkv,
                in_=dw_caqkv[l][:, D:3 * D].rearrange("(kc p) m -> p kc m", p=P),
            )
            wcaq = load_w(p_wout, dw_caqkv[l][:, 0:D], NDT, D, "wout")
            wcao = load_w(p_wout, dw_caout[l], NDT, D, "wout")
            h4 = layer_norm(y4, bf16, p_h, "h", stats=yst)
            bqkv = load_bias(db_caqkv[l], 12, "bias12")
            Qs = proj_fm(wcaq, 0, 4, h4, bqkv, 0, p_q, "q")
            Ks, Vs, weave = make_kv(wkv, 0, D, eob_all, bqkv, db_caqkv_bf[l], 4, 2 * D)
            OTs = attention(Qs, Ks, Vs, None, False, weave=weave)
            y4, yst = proj_residual(wcao, 0, NDT, OTs, db_caout[l], y4)

            # ffn
            h4 = layer_norm(y4, bf16, p_h, "h", stats=yst)
            y4, yst = ffn(dw_f1[l], dw_f2[l], db_f1[l], db_f2[l], h4, y4)

        dof = layer_norm(y4, f32, p_eo, "eof", stats=yst)
        for dt in range(NDT):
            nc.sync.dma_start(out=dec_part[dt], in_=dof[:, dt, :])

    nc.compile()
    return nc


def make_in_maps(inputs):
    inp = {k: np.asarray(v) for k, v in inputs.items()}
    f32 = np.float32

    W = {
        "ew_qkv": np.ascontiguousarray(inp["e_qkv_w"].swapaxes(1, 2)).astype(BF16),
        "ew_out": np.ascontiguousarray(inp["e_out_w"].swapaxes(1, 2)).astype(BF16),
        "ew_f1": np.ascontiguousarray(inp["e_ff1_w"].swapaxes(1, 2)).astype(BF16),
        "ew_f2": np.ascontiguousarray(inp["e_ff2_w"].swapaxes(1, 2)).astype(BF16),
        "eb_qkv": inp["e_qkv_b"].astype(f32).reshape(NE, 3 * D, 1),
        "eb_out": inp["e_out_b"].astype(f32).reshape(NE, D, 1),
        "eb_f1": inp["e_ff1_b"].astype(f32).reshape(NE, F, 1),
        "eb_f2": inp["e_ff2_b"].astype(f32).reshape(NE, D, 1),
        "dw_saqkv": np.ascontiguousarray(inp["d_sa_qkv_w"].swapaxes(1, 2)).astype(BF16),
        "db_saqkv": inp["d_sa_qkv_b"].astype(f32).reshape(ND, 3 * D, 1),
        "dw_saout": np.ascontiguousarray(inp["d_sa_out_w"].swapaxes(1, 2)).astype(BF16),
        "db_saout": inp["d_sa_out_b"].astype(f32).reshape(ND, D, 1),
        "dw_caqkv": np.ascontiguousarray(inp["d_ca_qkv_w"].swapaxes(1, 2)).astype(BF16),
        "db_caqkv": inp["d_ca_qkv_b"].astype(f32).reshape(ND, 3 * D, 1),
        "dw_caout": np.ascontiguousarray(inp["d_ca_out_w"].swapaxes(1, 2)).astype(BF16),
        "db_caout": inp["d_ca_out_b"].astype(f32).reshape(ND, D, 1),
        "dw_f1": np.ascontiguousarray(inp["d_ff1_w"].swapaxes(1, 2)).astype(BF16),
        "db_f1": inp["d_ff1_b"].astype(f32).reshape(ND, F, 1),
        "dw_f2": np.ascontiguousarray(inp["d_ff2_w"].swapaxes(1, 2)).astype(BF16),
        "db_f2": inp["d_ff2_b"].astype(f32).reshape(ND, D, 1),
        "eb_qkv_bf": inp["e_qkv_b"].astype(BF16).reshape(NE, 3 * D, 1),
        "db_saqkv_bf": inp["d_sa_qkv_b"].astype(BF16).reshape(ND, 3 * D, 1),
        "db_caqkv_bf": inp["d_ca_qkv_b"].astype(BF16).reshape(ND, 3 * D, 1),
    }

    in_maps = []
    for c in range(NCORE):
        b, half = c // 2, c % 2
        sl = slice(half * TOWN, (half + 1) * TOWN)
        m = dict(W)
        xT = np.ascontiguousarray(inp["enc_in"][b].astype(f32).T[:, sl])
        m["x0T"] = xT.reshape(NDT, P, TOWN)
        yT = np.ascontiguousarray(inp["dec_in"][b].astype(f32).T[:, sl])
        m["y0T"] = yT.reshape(NDT, P, TOWN)
        xyz = inp["enc_xyz"][b].astype(f32)
        n2 = (xyz * xyz).sum(-1, dtype=f32).astype(f32)
        xq2 = (np.float32(2.0) * xyz[sl]).astype(f32)
        m["xq2"] = np.ascontiguousarray(xq2)
        m["xq2row"] = np.ascontiguousarray(xq2.T)
        xkn = np.concatenate([xyz, n2[:, None]], 1).astype(f32)
        m["xkn"] = np.ascontiguousarray(xkn)
        m["xrow"] = np.ascontiguousarray(xkn.T)
        bos = np.full((1, TOWN), 1e30, f32)
        if half == 0:
            bos[0, 0] = NEG
        m["bosrow"] = bos
        qg = np.arange(half * TOWN, (half + 1) * TOWN)
        kg = np.arange(LE)
        m["causal"] = np.ascontiguousarray(
            (kg[:, None] <= qg[None, :]).astype(BF16)
        ).reshape(NKT, P, TOWN)
        in_maps.append(m)
    return in_maps


def assemble(results):
    enc = np.zeros((B, LE, D), np.float32)
    dec = np.zeros((B, LD, D), np.float32)
    for c, r in enumerate(results):
        b, half = c // 2, c % 2
        sl = slice(half * TOWN, (half + 1) * TOWN)
        enc[b, sl, :] = r["enc_part"].reshape(D, TOWN).T
        dec[b, sl, :] = r["dec_part"].reshape(D, TOWN).T
    return enc, dec


def kernel(**inputs):
    from concourse import bass_utils

    if "nc" not in _CACHE:
        _CACHE["nc"] = build()
    nc = _CACHE["nc"]
    in_maps = make_in_maps(inputs)
    res = bass_utils.run_bass_kernel_spmd(
        nc, in_maps, core_ids=list(range(NCORE))
    )
    return assemble(res.results)
